# revision 26
# baseline (speedup 1.0000x reference)
"""Criss-cross (CCNet) attention kernel for Trainium2, 8 NeuronCores.

Sharding: core c in 0..7 -> batch b = c//2, value-channel half h = c%2.
Each core computes, for its (b, h): the full joint row+column softmax
attention with 256 of the 512 value/output channels.

Device-side math (per core), H = W = 128, Cqk = 64, Cv = 256:
  - q,k = Wq x, Wk x   (fp16 matmuls)
  - row pass, per row y:  E[i,x] = sum_c k[c,y,i] q[c,y,x]; P = exp(E)
      O[x, c] = P^T V_y (unnormalized bf16), s = ones^T P (one extra
      N=512 matmul per chunk with a [128,1] ones stationary)
  - col pass, per col x:  E[j,y] = sum_c k[c,j,x] q[c,y,x]; P = exp(E)
      with diagonal j==y masked to 0 (-6e4*I fp16 matmul); same shape.
  - v staged to DRAM (bf16) between the passes.
Host combines:  out = (O_r + O_c) / (s_r + s_c)  in fp32.

Precision: projections/energies fp16 (fp32 PSUM accum); P, V and the
unnormalized outputs bf16 (P needs fp32-range exponent: |e|~50 so exp
overflows fp16); sums fp32. End-to-end rel err ~6e-3.

Performance notes (per core, HW-profiled):
  - DMA descriptor size decides the column pass's pace: V slots are
    kept contiguous 256-wide (no interleaved ones columns) so the
    column gather moves 4 KiB per descriptor, and orow/ocol are stored
    x-major/y-major so output writes run 2-4 KiB contiguous per
    partition line. With 512 B descriptors the column pass was
    DMA-paced and the PE clock-gate (HAM) froze at half rate.
  - softmax normalization happens on host from the exported fp32 sums
    (frees ~75us of scalar/vector evacuation + reciprocal work).
  - PSUM->SBUF evacuations are one contiguous [128, 512] bf16 copy per
    PSUM bank, alternating scalar/vector engines.
  - a burst of dependency-free filler matmuls bridges the row->col
    transition: the first column gather waits on the last vscr write,
    and a >3.4us PE idle gap would re-throttle the PE clock to 1.2 GHz
    for the entire column pass (re-warming needs ~3.4us of sustained
    uninterrupted busy, which a pipelined kernel never produces).
  - within one PSUM bank, all matmuls must share tile_position: mixing
    partition-base-0 and base-64 ops on the same bank wedges the device
    (HW-bisected; independent start/stop singles on different banks are
    fine). So all matmuls here keep base 0.
"""

import numpy as np

import concourse.tile as tile
from concourse import bacc, mybir
from concourse.bass_utils import run_bass_kernel_spmd

B, C, H, W = 4, 512, 128, 128
CQK = C // 8          # 64
CV = C // 2           # 256 v channels per core
HW = H * W
N_CORES = 8

F32 = mybir.dt.float32
F16 = mybir.dt.float16
BF16 = mybir.dt.bfloat16
EXP = mybir.ActivationFunctionType.Exp
COPY = mybir.ActivationFunctionType.Copy

_CACHE = {}


def _build(with_bias):
    nc = bacc.Bacc("TRN2", target_bir_lowering=False, debug=False,
                   num_devices=N_CORES)
    nck = 5 if with_bias else 4   # contraction chunks (last is the bias rows)
    xrows = C + (2 if with_bias else 0)

    xin = nc.dram_tensor("xin", [xrows, HW], F16, kind="ExternalInput").ap()
    wqk = nc.dram_tensor("wqk", [xrows, 128], F16, kind="ExternalInput").ap()
    wv = nc.dram_tensor("wv", [xrows, CV], F16, kind="ExternalInput").ap()
    negid = nc.dram_tensor("negid", [128, 128], F16,
                           kind="ExternalInput").ap()
    id4 = nc.dram_tensor("id4", [128, 512], F16, kind="ExternalInput").ap()

    vscr = nc.dram_tensor("vscr", [HW, CV], BF16).ap()
    # orow rows are x-major (row index = x*128 + y), ocol rows y-major
    # (row index = y*128 + x): each pass's store then writes 2-4 KiB
    # contiguous runs per partition line.
    orow = nc.dram_tensor("orow", [HW, CV], BF16, kind="ExternalOutput").ap()
    ocol = nc.dram_tensor("ocol", [HW, CV], BF16, kind="ExternalOutput").ap()
    srow = nc.dram_tensor("srow", [1, HW], F32, kind="ExternalOutput").ap()
    scol = nc.dram_tensor("scol", [1, HW], F32, kind="ExternalOutput").ap()

    with tile.TileContext(nc) as tc:
        with (
            tc.tile_pool(name="cst", bufs=1) as cst,
            tc.tile_pool(name="xs", bufs=3) as xsp,
            tc.tile_pool(name="p4", bufs=8) as p4p,
            tc.tile_pool(name="o16", bufs=2) as o16p,
            tc.tile_pool(name="sr", bufs=2) as srp,
            tc.tile_pool(name="psbig", bufs=2, space="PSUM") as psbigp,
            tc.tile_pool(name="psv", bufs=2, space="PSUM") as psvp,
            tc.tile_pool(name="psO", bufs=3, space="PSUM") as psOp,
        ):
            # x streams as [128, 1024] pair-chunks (chunks 2p, 2p+1) so each
            # partition line is 2 KiB. Prefetch the first two pairs first.
            xpre = {}
            for p in range(2):
                xs0 = []
                for k in range(nck):
                    rows = 128 if k < 4 else 2
                    xt = xsp.tile([128, 1024], F16, tag=f"xs{k}",
                                  name=f"xpre{p}_{k}")
                    nc.sync.dma_start(
                        xt[0:rows, :],
                        xin[k * 128:k * 128 + rows,
                            p * 1024:(p + 1) * 1024])
                    xs0.append(xt)
                xpre[p] = xs0

            # ---- persistent constants / accumulators ----
            WQK = cst.tile([128, nck * 128], F16)
            for k in range(nck):
                rows = 128 if k < 4 else 2
                nc.sync.dma_start(WQK[0:rows, k * 128:k * 128 + 128],
                                  wqk[k * 128:k * 128 + rows, :])
            WV = cst.tile([128, nck * CV], F16)
            for k in range(nck):
                rows = 128 if k < 4 else 2
                nc.sync.dma_start(WV[0:rows, k * CV:k * CV + CV],
                                  wv[k * 128:k * 128 + rows, :])
            NEGID = cst.tile([128, 128], F16)
            nc.sync.dma_start(NEGID[:], negid[:])
            ID4 = cst.tile([128, 512], F16)
            nc.sync.dma_start(ID4[:], id4[:])
            QD = cst.tile([64, HW], F16)
            KD = cst.tile([128, HW], F16)

            # consolidated V buffers, contiguous 256-wide slots.
            # VTB has 4 groups of 8 slots so column gathers prefetch deep.
            VB = cst.tile([128, 8 * CV], BF16)
            VTB = cst.tile([128, 32 * CV], BF16)

            qd_of = QD[:].rearrange("c (y x) -> c y x", x=128)
            kd_of = KD[0:64, :].rearrange("c (y x) -> c y x", x=128)
            vscr_row4 = vscr.rearrange("(g t x) c -> g x t c", t=4, x=128)
            orow_4 = orow.rearrange("(x g t) c -> g x t c", t=4, g=32)
            ocol_8 = ocol.rearrange("(y g t) c -> g y t c", t=8, g=16)
            srow_4 = srow.rearrange("o (g n) -> o g n", n=2048)
            scol_4 = scol.rearrange("o (g n) -> o g n", n=2048)

            # =================== phase A + row pass ===================
            # software pipeline: chunk ch's aggregation tail is emitted
            # after chunk ch+1's head so exp/evac queueing never stalls PE
            row_state = {}

            def row_head(ch):
                csl = slice(ch * 512, (ch + 1) * 512)
                xoff = (ch % 2) * 512
                xsub = slice(xoff, xoff + 512)
                vbase = (ch % 2) * 4
                xs = row_state[("xs", ch // 2)]
                # qk projection for these 512 pixels
                pqk = psbigp.tile([128, 512], F32, tag="psbig")
                for k in range(nck):
                    rows = 128 if k < 4 else 2
                    nc.tensor.matmul(pqk[:],
                                     WQK[0:rows, k * 128:(k + 1) * 128],
                                     xs[k][0:rows, xsub],
                                     start=(k == 0), stop=(k == nck - 1))
                nc.scalar.activation(QD[:, csl], pqk[0:64, :], COPY)
                nc.scalar.activation(KD[64:128, csl], pqk[64:128, :], COPY)
                nc.vector.tensor_copy(KD[0:64, csl], KD[64:128, csl])

                pE = psbigp.tile([128, 512], F32, tag="psbig")
                # v projection, two rows per PSUM bank
                for pair in range(2):
                    pv = psvp.tile([128, 512], F32)
                    for q2 in range(2):
                        yy = pair * 2 + q2
                        xsl = slice(xoff + yy * 128, xoff + yy * 128 + 128)
                        for k in range(nck):
                            rows = 128 if k < 4 else 2
                            nc.tensor.matmul(
                                pv[:, q2 * 256:q2 * 256 + 256],
                                xs[k][0:rows, xsl],
                                WV[0:rows, k * CV:(k + 1) * CV],
                                start=(k == 0), stop=(k == nck - 1))
                    nc.vector.tensor_copy(
                        VB[:, (vbase + pair * 2) * CV:
                           (vbase + pair * 2 + 2) * CV],
                        pv[:])
                for yy in range(4):
                    y = ch * 4 + yy
                    ysl = slice(y * 128, (y + 1) * 128)
                    # row energies E[i, x]
                    nc.tensor.matmul(pE[:, yy * 128:(yy + 1) * 128],
                                     KD[0:64, ysl], QD[:, ysl],
                                     start=True, stop=True)
                # batched v write: 4 rows at once
                nc.sync.dma_start(
                    vscr_row4[ch],
                    VB[:, vbase * CV:(vbase + 4) * CV].rearrange(
                        "p (t c) -> p t c", c=CV))
                p4 = p4p.tile([128, 512], BF16)
                nc.scalar.activation(p4[:], pE[:], EXP)
                # row sums on the otherwise-idle gpsimd engine
                if ch % 4 == 0:
                    row_state[("sr", ch // 4)] = srp.tile(
                        [1, 2048], F32, tag="sr", name=f"sr{ch // 4}")
                sr = row_state[("sr", ch // 4)]
                nc.gpsimd.tensor_reduce(
                    sr[:, (ch % 4) * 512:(ch % 4) * 512 + 512], p4[:],
                    axis=mybir.AxisListType.C, op=mybir.AluOpType.add)
                row_state[ch] = p4
                if ch % 2 == 1:
                    del row_state[("xs", ch // 2)]

            def row_tail(ch):
                vbase = (ch % 2) * 4
                p4 = row_state.pop(ch)
                o16 = o16p.tile([128, 1024], BF16, tag="o16r")
                for half in range(2):
                    pO = psOp.tile([128, 512], F32)
                    for q2 in range(2):
                        yy = half * 2 + q2
                        ysl = slice(yy * 128, (yy + 1) * 128)
                        nc.tensor.matmul(
                            pO[:, q2 * 256:q2 * 256 + 256],
                            p4[:, ysl],
                            VB[:, (vbase + yy) * CV:(vbase + yy + 1) * CV],
                            start=True, stop=True)
                    if half == 0:
                        nc.scalar.activation(o16[:, 0:512], pO[:], COPY)
                    else:
                        nc.vector.tensor_copy(o16[:, 512:1024], pO[:])
                nc.sync.dma_start(
                    orow_4[ch],
                    o16[:].rearrange("p (t c) -> p t c", c=CV))
                if ch % 4 == 3:
                    sr = row_state.pop(("sr", ch // 4))
                    nc.sync.dma_start(srow_4[:, ch // 4, :], sr[:])

            def load_x(p):
                xs = []
                for k in range(nck):
                    rows = 128 if k < 4 else 2
                    xt = xsp.tile([128, 1024], F16, tag=f"xs{k}")
                    nc.sync.dma_start(
                        xt[0:rows, :],
                        xin[k * 128:k * 128 + rows,
                            p * 1024:(p + 1) * 1024])
                    xs.append(xt)
                row_state[("xs", p)] = xs

            row_state[("xs", 0)] = xpre[0]
            row_state[("xs", 1)] = xpre[1]
            for ch in range(33):
                if ch < 32:
                    if ch % 2 == 0 and ch + 4 < 32:
                        load_x((ch + 4) // 2)
                    row_head(ch)
                if ch >= 1:
                    row_tail(ch - 1)

            # =================== column pass ===================
            vscr_col8 = vscr.rearrange("(j g t) c -> g j t c", t=8, j=128)
            col_state = {}

            def col_gather(sch):
                g = (sch % 4) * 8
                nc.sync.dma_start(
                    VTB[:, g * CV:(g + 8) * CV].rearrange(
                        "p (t c) -> p t c", c=CV),
                    vscr_col8[sch])

            def col_head(sch):
                p4s = []
                if sch % 2 == 0:
                    col_state[("sc", sch // 2)] = srp.tile(
                        [1, 2048], F32, tag="sr", name=f"sc{sch // 2}")
                sc = col_state[("sc", sch // 2)]
                for g in range(2):
                    pE = psbigp.tile([128, 512], F32, tag="psbig")
                    for xx in range(4):
                        x = sch * 8 + g * 4 + xx
                        nc.tensor.matmul(pE[:, xx * 128:(xx + 1) * 128],
                                         kd_of[:, :, x], qd_of[:, :, x],
                                         start=(xx == 0), stop=False)
                    # mask the j==y diagonal of all 4 tiles
                    nc.tensor.matmul(pE[:], NEGID[:], ID4[:],
                                     start=False, stop=True)
                    p4 = p4p.tile([128, 512], BF16, tag="p4c")
                    nc.scalar.activation(p4[:], pE[:], EXP)
                    qo = ((sch % 2) * 2 + g) * 512
                    nc.gpsimd.tensor_reduce(
                        sc[:, qo:qo + 512], p4[:],
                        axis=mybir.AxisListType.C, op=mybir.AluOpType.add)
                    p4s.append(p4)
                col_state[sch] = p4s

            def col_tail(sch):
                vbase = (sch % 4) * 8
                p4s = col_state.pop(sch)
                o16 = o16p.tile([128, 2048], BF16, tag="o16c")
                for g in range(2):
                    p4 = p4s[g]
                    for half in range(2):
                        pO = psOp.tile([128, 512], F32)
                        for q2 in range(2):
                            xx = half * 2 + q2
                            slot = vbase + g * 4 + xx
                            nc.tensor.matmul(
                                pO[:, q2 * 256:q2 * 256 + 256],
                                p4[:, xx * 128:(xx + 1) * 128],
                                VTB[:, slot * CV:(slot + 1) * CV],
                                start=True, stop=True)
                        oco = (g * 2 + half) * 512
                        if (g * 2 + half) % 2 == 0:
                            nc.scalar.activation(
                                o16[:, oco:oco + 512], pO[:], COPY)
                        else:
                            nc.vector.tensor_copy(
                                o16[:, oco:oco + 512], pO[:])
                nc.sync.dma_start(
                    ocol_8[sch],
                    o16[:].rearrange("p (t c) -> p t c", c=CV))
                if sch % 2 == 1:
                    sc = col_state.pop(("sc", sch // 2))
                    nc.sync.dma_start(scol_4[:, sch // 2, :], sc[:])

            col_gather(0)
            col_gather(1)
            # HAM keep-warm: the first column gather can only start after
            # the last vscr write lands, a ~5us PE-idle chain that would
            # re-throttle the PE clock to 1.2 GHz for the entire column
            # pass. Bridge the gap with dependency-free filler matmuls.
            for f in range(16):
                pF = psbigp.tile([128, 512], F32, tag="psbig")
                nc.tensor.matmul(pF[:], NEGID[:], ID4[:],
                                 start=True, stop=True)
                nc.tensor.matmul(pF[:], NEGID[:], ID4[:],
                                 start=True, stop=True)
            for sch in range(17):
                if sch < 16:
                    if sch + 2 < 16:
                        col_gather(sch + 2)
                    col_head(sch)
                if sch >= 1:
                    col_tail(sch - 1)

    nc.compile()
    return nc


def _get_nc(with_bias):
    key = bool(with_bias)
    if key not in _CACHE:
        _CACHE[key] = _build(key)
    return _CACHE[key]


def kernel(x, Wq, bq, Wk, bk, Wv, bv, _trace=False, _raw=False):
    x = np.asarray(x, np.float32)
    Wq = np.asarray(Wq, np.float32)
    Wk = np.asarray(Wk, np.float32)
    Wv = np.asarray(Wv, np.float32)
    bq = np.asarray(bq, np.float32)
    bk = np.asarray(bk, np.float32)
    bv = np.asarray(bv, np.float32)

    with_bias = bool(np.any(bq) or np.any(bk) or np.any(bv))
    nc = _get_nc(with_bias)

    import ml_dtypes
    negid_a = np.ascontiguousarray(
        (-6e4 * np.eye(128)).astype(np.float16))
    id4_a = np.ascontiguousarray(
        np.tile(np.eye(128), (1, 4)).astype(np.float16))
    wqk_full = np.concatenate([Wq.T, Wk.T], axis=1)       # [C, 128]
    if with_bias:
        bias_qk = np.concatenate([bq, bk])[None, :]       # [1, 128]
        wqk_full = np.concatenate(
            [wqk_full, bias_qk, np.zeros_like(bias_qk)], axis=0)
    wqk_full = wqk_full.astype(np.float16)

    in_maps = []
    for core in range(N_CORES):
        b, h = core // 2, core % 2
        xb = x[b].reshape(C, HW)
        wvh = Wv[h * CV:(h + 1) * CV, :].T                # [C, CV]
        if with_bias:
            xb = np.concatenate([xb, np.ones((1, HW), np.float32),
                                 np.zeros((1, HW), np.float32)], axis=0)
            bvh = bv[h * CV:(h + 1) * CV][None, :]
            wvh = np.concatenate([wvh, bvh, np.zeros_like(bvh)], axis=0)
        xb = np.ascontiguousarray(xb.astype(np.float16))
        wvh = np.ascontiguousarray(wvh.astype(np.float16))
        in_maps.append({
            "xin": xb, "wqk": wqk_full, "wv": wvh,
            "negid": negid_a, "id4": id4_a,
        })

    res = run_bass_kernel_spmd(nc, in_maps, list(range(N_CORES)),
                               trace=bool(_trace))
    if _raw:
        return res

    out = np.empty((B, C, H, W), np.float32)
    for core in range(N_CORES):
        b, h = core // 2, core % 2
        r = res.results[core]
        o_r = r["orow"].astype(np.float32).reshape(W, H, CV)   # [x, y, c]
        o_c = r["ocol"].astype(np.float32).reshape(H, W, CV)   # [y, x, c]
        s_r = r["srow"].reshape(H, W)                          # [y, x]
        s_c = r["scol"].reshape(W, H).T                        # [y, x]
        comb = (o_r.transpose(1, 0, 2) + o_c) / (s_r + s_c)[:, :, None]
        out[b, h * CV:(h + 1) * CV] = comb.transpose(2, 0, 1)

    if _trace:
        return out, res
    return out


# revision 27
# speedup vs baseline: 16.9930x; 16.9930x over previous
"""Criss-cross (CCNet) attention kernel for Trainium2, 8 NeuronCores.

Sharding: core c in 0..7 -> batch b = c//2, value-channel half h = c%2.
Each core computes, for its (b, h): the full joint row+column softmax
attention with 256 of the 512 value/output channels.

Device-side math (per core), H = W = 128, Cqk = 64, Cv = 256:
  - q,k = Wq x, Wk x   (fp16 matmuls)
  - row pass, per row y:  E[i,x] = sum_c k[c,y,i] q[c,y,x]; P = exp(E)
      O[x, 0:256] = P^T V_y, O[x, 256:258] = row-sums of P (two ones
      columns appended to each V slot)
      orow[x*128+y, :] = [O | sums]  (unnormalized bf16, x-major rows)
  - col pass, per col x:  E[j,y] = sum_c k[c,j,x] q[c,y,x]; P = exp(E)
      with diagonal j==y masked to 0 (-6e4*I fp16 matmul); same
      aggregation; ocol rows y-major.
  - v staged to DRAM (bf16) between the passes.
Host combines:  out = (O_r + O_c) / (s_r + s_c)  in fp32.

Precision: projections/energies fp16 (fp32 PSUM accumulation); P, V,
outputs and sums bf16 (P needs fp32-range exponent: energies reach
|e|~50 so exp overflows fp16's range). End-to-end rel err ~6.5e-3.

Performance notes (per core, HW-profiled):
  - 16-bit matmuls stream 1 cyc/row at 2.4 GHz; each matmul also pays
    a ~150ns serial LDWEIGHTS+drain (the PE only hides weight loads
    across disjoint row groups, and mixing tile_positions within one
    PSUM bank wedges the device - HW-bisected - so all matmuls here
    keep partition base 0).
  - softmax runs unnormalized; row-sums ride along as two appended
    ones-columns in the aggregation matmul; normalization happens on
    host (frees ~75us of scalar/vector evacuation + reciprocal work).
  - PSUM->SBUF evacuations batched: one strided [128, 2, 258] bf16
    activation per 2-bank PSUM tile, alternating scalar/vector.
  - orow is stored x-major and ocol y-major so each pass's output DMA
    writes 2-4 KiB contiguous runs per partition line (small DMA
    descriptors otherwise pace the column pass and freeze the HAM
    PE clock-gate at half rate).
  - column gathers prefetch two super-chunks ahead into a 4-group
    transposed-V buffer.
"""

import numpy as np

import concourse.tile as tile
from concourse import bacc, mybir
from concourse.bass_utils import run_bass_kernel_spmd

B, C, H, W = 4, 512, 128, 128
CQK = C // 8          # 64
CV = C // 2           # 256 v channels per core
HW = H * W
N_CORES = 8

F32 = mybir.dt.float32
F16 = mybir.dt.float16
BF16 = mybir.dt.bfloat16
EXP = mybir.ActivationFunctionType.Exp
COPY = mybir.ActivationFunctionType.Copy

_CACHE = {}

VBW = 258   # v buffer width: 256 channels + 2 ones columns


def _build(with_bias):
    nc = bacc.Bacc("TRN2", target_bir_lowering=False, debug=False,
                   num_devices=N_CORES)
    nck = 5 if with_bias else 4   # contraction chunks (last is the bias rows)
    xrows = C + (2 if with_bias else 0)

    xin = nc.dram_tensor("xin", [xrows, HW], F16, kind="ExternalInput").ap()
    wqk = nc.dram_tensor("wqk", [xrows, 128], F16, kind="ExternalInput").ap()
    wv = nc.dram_tensor("wv", [xrows, CV], F16, kind="ExternalInput").ap()
    negid = nc.dram_tensor("negid", [128, 128], F16,
                           kind="ExternalInput").ap()
    id4 = nc.dram_tensor("id4", [128, 512], F16, kind="ExternalInput").ap()
    ones2 = nc.dram_tensor("ones2", [128, 64], BF16, kind="ExternalInput").ap()

    vscr = nc.dram_tensor("vscr", [HW, CV], BF16).ap()
    orow = nc.dram_tensor("orow", [HW, VBW], BF16, kind="ExternalOutput").ap()
    ocol = nc.dram_tensor("ocol", [HW, VBW], BF16, kind="ExternalOutput").ap()

    with tile.TileContext(nc) as tc:
        with (
            tc.tile_pool(name="cst", bufs=1) as cst,
            tc.tile_pool(name="xs", bufs=3) as xsp,
            tc.tile_pool(name="p4", bufs=8) as p4p,
            tc.tile_pool(name="o16", bufs=2) as o16p,
            tc.tile_pool(name="psbig", bufs=2, space="PSUM") as psbigp,
            tc.tile_pool(name="psv", bufs=2, space="PSUM") as psvp,
            tc.tile_pool(name="psO", bufs=2, space="PSUM") as psOp,
        ):
            # x streams as [128, 1024] pair-chunks (chunks 2p, 2p+1) so each
            # partition line is 2 KiB. Prefetch the first two pairs first.
            xpre = {}
            for p in range(2):
                xs0 = []
                for k in range(nck):
                    rows = 128 if k < 4 else 2
                    xt = xsp.tile([128, 1024], F16, tag=f"xs{k}",
                                  name=f"xpre{p}_{k}")
                    nc.sync.dma_start(
                        xt[0:rows, :],
                        xin[k * 128:k * 128 + rows,
                            p * 1024:(p + 1) * 1024])
                    xs0.append(xt)
                xpre[p] = xs0

            # ---- persistent constants / accumulators ----
            WQK = cst.tile([128, nck * 128], F16)
            for k in range(nck):
                rows = 128 if k < 4 else 2
                nc.sync.dma_start(WQK[0:rows, k * 128:k * 128 + 128],
                                  wqk[k * 128:k * 128 + rows, :])
            WV = cst.tile([128, nck * CV], F16)
            for k in range(nck):
                rows = 128 if k < 4 else 2
                nc.sync.dma_start(WV[0:rows, k * CV:k * CV + CV],
                                  wv[k * 128:k * 128 + rows, :])
            NEGID = cst.tile([128, 128], F16)
            nc.sync.dma_start(NEGID[:], negid[:])
            ID4 = cst.tile([128, 512], F16)
            nc.sync.dma_start(ID4[:], id4[:])
            QD = cst.tile([64, HW], F16)
            KD = cst.tile([128, HW], F16)

            # consolidated V buffers: slots of [128, 258], ones columns
            # (256:258 of each slot) loaded once. VTB has 4 groups of 8
            # slots so column gathers can run two pipeline stages ahead.
            VB = cst.tile([128, 8 * VBW], BF16)
            VTB = cst.tile([128, 32 * VBW], BF16)
            nc.sync.dma_start(
                VB[:].rearrange("p (s w) -> p s w", w=VBW)[:, :, 256:258],
                ones2[:, 0:16].rearrange("p (s w) -> p s w", w=2))
            nc.sync.dma_start(
                VTB[:].rearrange("p (s w) -> p s w", w=VBW)[:, :, 256:258],
                ones2[:].rearrange("p (s w) -> p s w", w=2))

            qd_of = QD[:].rearrange("c (y x) -> c y x", x=128)
            kd_of = KD[0:64, :].rearrange("c (y x) -> c y x", x=128)
            vscr_row4 = vscr.rearrange("(g t x) c -> g x t c", t=4, x=128)
            # orow rows x-major (row = x*128 + y), ocol rows y-major
            # (row = y*128 + x): output stores write contiguous runs.
            orow_4 = orow.rearrange("(x g t) c -> g x t c", t=4, g=32)
            ocol_8 = ocol.rearrange("(y g t) c -> g y t c", t=8, g=16)
            vb_slots = VB[:].rearrange("p (s w) -> p s w", w=VBW)
            vtb_slots = VTB[:].rearrange("p (s w) -> p s w", w=VBW)

            # =================== phase A + row pass ===================
            # software pipeline: chunk ch's aggregation tail is emitted
            # after chunk ch+1's head so exp/evac queueing never stalls PE
            row_state = {}

            def row_head(ch):
                csl = slice(ch * 512, (ch + 1) * 512)
                xoff = (ch % 2) * 512
                xsub = slice(xoff, xoff + 512)
                vbase = (ch % 2) * 4
                xs = row_state[("xs", ch // 2)]
                # qk projection for these 512 pixels
                pqk = psbigp.tile([128, 512], F32, tag="psbig")
                for k in range(nck):
                    rows = 128 if k < 4 else 2
                    nc.tensor.matmul(pqk[:],
                                     WQK[0:rows, k * 128:(k + 1) * 128],
                                     xs[k][0:rows, xsub],
                                     start=(k == 0), stop=(k == nck - 1))
                nc.scalar.activation(QD[:, csl], pqk[0:64, :], COPY)
                nc.scalar.activation(KD[64:128, csl], pqk[64:128, :], COPY)
                nc.vector.tensor_copy(KD[0:64, csl], KD[64:128, csl])

                pE = psbigp.tile([128, 512], F32, tag="psbig")
                # v projection, two rows per PSUM bank
                for pair in range(2):
                    pv = psvp.tile([128, 512], F32)
                    for q2 in range(2):
                        yy = pair * 2 + q2
                        xsl = slice(xoff + yy * 128, xoff + yy * 128 + 128)
                        for k in range(nck):
                            rows = 128 if k < 4 else 2
                            nc.tensor.matmul(
                                pv[:, q2 * 256:q2 * 256 + 256],
                                xs[k][0:rows, xsl],
                                WV[0:rows, k * CV:(k + 1) * CV],
                                start=(k == 0), stop=(k == nck - 1))
                    nc.vector.tensor_copy(
                        vb_slots[:, vbase + pair * 2:vbase + pair * 2 + 2,
                                 0:256],
                        pv[:].rearrange("p (b c) -> p b c", c=256))
                for yy in range(4):
                    y = ch * 4 + yy
                    ysl = slice(y * 128, (y + 1) * 128)
                    # row energies E[i, x]
                    nc.tensor.matmul(pE[:, yy * 128:(yy + 1) * 128],
                                     KD[0:64, ysl], QD[:, ysl],
                                     start=True, stop=True)
                # batched v write: 4 rows at once
                nc.sync.dma_start(
                    vscr_row4[ch],
                    vb_slots[:, vbase:vbase + 4, 0:256])
                p4 = p4p.tile([128, 512], BF16)
                nc.scalar.activation(p4[:], pE[:], EXP)
                row_state[ch] = p4
                if ch % 2 == 1:
                    del row_state[("xs", ch // 2)]

            def row_tail(ch):
                vbase = (ch % 2) * 4
                p4 = row_state.pop(ch)
                o16 = o16p.tile([128, 1032], BF16, tag="o16r")
                for half in range(2):
                    pO = psOp.tile([128, 1024], F32)
                    for q2 in range(2):
                        yy = half * 2 + q2
                        osl = slice(q2 * 512, q2 * 512 + VBW)
                        ysl = slice(yy * 128, (yy + 1) * 128)
                        sl = vb_slots[:, vbase + yy:vbase + yy + 1, :]
                        nc.tensor.matmul(pO[:, osl], p4[:, ysl],
                                         sl[:, 0, :],
                                         start=True, stop=True)
                    dst = o16[:, half * 516:half * 516 + 516].rearrange(
                        "p (b w) -> p b w", w=VBW)
                    src = pO[:].rearrange("p (b k) -> p b k",
                                          k=512)[:, :, 0:VBW]
                    if half == 0:
                        nc.scalar.activation(dst, src, COPY)
                    else:
                        nc.vector.tensor_copy(dst, src)
                nc.sync.dma_start(
                    orow_4[ch],
                    o16[:].rearrange("p (t c) -> p t c", c=VBW))

            def load_x(p):
                xs = []
                for k in range(nck):
                    rows = 128 if k < 4 else 2
                    xt = xsp.tile([128, 1024], F16, tag=f"xs{k}")
                    nc.sync.dma_start(
                        xt[0:rows, :],
                        xin[k * 128:k * 128 + rows,
                            p * 1024:(p + 1) * 1024])
                    xs.append(xt)
                row_state[("xs", p)] = xs

            row_state[("xs", 0)] = xpre[0]
            row_state[("xs", 1)] = xpre[1]
            for ch in range(33):
                if ch < 32:
                    if ch % 2 == 0 and ch + 4 < 32:
                        load_x((ch + 4) // 2)
                    row_head(ch)
                if ch >= 1:
                    row_tail(ch - 1)

            # =================== column pass ===================
            vscr_col8 = vscr.rearrange("(j g t) c -> g j t c", t=8, j=128)
            col_state = {}

            def col_gather(sch):
                g = (sch % 4) * 8
                nc.sync.dma_start(vtb_slots[:, g:g + 8, 0:256],
                                  vscr_col8[sch])

            def col_head(sch):
                p4s = []
                for g in range(2):
                    pE = psbigp.tile([128, 512], F32, tag="psbig")
                    for xx in range(4):
                        x = sch * 8 + g * 4 + xx
                        nc.tensor.matmul(pE[:, xx * 128:(xx + 1) * 128],
                                         kd_of[:, :, x], qd_of[:, :, x],
                                         start=(xx == 0), stop=False)
                    # mask the j==y diagonal of all 4 tiles
                    nc.tensor.matmul(pE[:], NEGID[:], ID4[:],
                                     start=False, stop=True)
                    p4 = p4p.tile([128, 512], BF16, tag="p4c")
                    nc.scalar.activation(p4[:], pE[:], EXP)
                    p4s.append(p4)
                col_state[sch] = p4s

            def col_tail(sch):
                vbase = (sch % 4) * 8
                p4s = col_state.pop(sch)
                o16 = o16p.tile([128, 2064], BF16, tag="o16c")
                for g in range(2):
                    p4 = p4s[g]
                    for half in range(2):
                        pO = psOp.tile([128, 1024], F32)
                        for q2 in range(2):
                            xx = half * 2 + q2
                            slot = vbase + g * 4 + xx
                            osl = slice(q2 * 512, q2 * 512 + VBW)
                            xsl = slice(xx * 128, (xx + 1) * 128)
                            sl = vtb_slots[:, slot:slot + 1, :]
                            nc.tensor.matmul(pO[:, osl], p4[:, xsl],
                                             sl[:, 0, :],
                                             start=True, stop=True)
                        oco = (g * 2 + half) * 516
                        dst = o16[:, oco:oco + 516].rearrange(
                            "p (b w) -> p b w", w=VBW)
                        src = pO[:].rearrange("p (b k) -> p b k",
                                              k=512)[:, :, 0:VBW]
                        if (g * 2 + half) % 2 == 0:
                            nc.scalar.activation(dst, src, COPY)
                        else:
                            nc.vector.tensor_copy(dst, src)
                nc.sync.dma_start(
                    ocol_8[sch],
                    o16[:].rearrange("p (t c) -> p t c", c=VBW))

            col_gather(0)
            col_gather(1)
            for sch in range(17):
                if sch < 16:
                    if sch + 2 < 16:
                        col_gather(sch + 2)
                    col_head(sch)
                if sch >= 1:
                    col_tail(sch - 1)

    nc.compile()
    return nc


def _get_nc(with_bias):
    key = bool(with_bias)
    if key not in _CACHE:
        _CACHE[key] = _build(key)
    return _CACHE[key]


def kernel(x, Wq, bq, Wk, bk, Wv, bv, _trace=False, _raw=False):
    x = np.asarray(x, np.float32)
    Wq = np.asarray(Wq, np.float32)
    Wk = np.asarray(Wk, np.float32)
    Wv = np.asarray(Wv, np.float32)
    bq = np.asarray(bq, np.float32)
    bk = np.asarray(bk, np.float32)
    bv = np.asarray(bv, np.float32)

    with_bias = bool(np.any(bq) or np.any(bk) or np.any(bv))
    nc = _get_nc(with_bias)

    import ml_dtypes
    negid_a = np.ascontiguousarray(
        (-6e4 * np.eye(128)).astype(np.float16))
    id4_a = np.ascontiguousarray(
        np.tile(np.eye(128), (1, 4)).astype(np.float16))
    ones2 = np.ones((128, 64), ml_dtypes.bfloat16)
    wqk_full = np.concatenate([Wq.T, Wk.T], axis=1)       # [C, 128]
    if with_bias:
        bias_qk = np.concatenate([bq, bk])[None, :]       # [1, 128]
        wqk_full = np.concatenate(
            [wqk_full, bias_qk, np.zeros_like(bias_qk)], axis=0)
    wqk_full = wqk_full.astype(np.float16)

    in_maps = []
    for core in range(N_CORES):
        b, h = core // 2, core % 2
        xb = x[b].reshape(C, HW)
        wvh = Wv[h * CV:(h + 1) * CV, :].T                # [C, CV]
        if with_bias:
            xb = np.concatenate([xb, np.ones((1, HW), np.float32),
                                 np.zeros((1, HW), np.float32)], axis=0)
            bvh = bv[h * CV:(h + 1) * CV][None, :]
            wvh = np.concatenate([wvh, bvh, np.zeros_like(bvh)], axis=0)
        xb = np.ascontiguousarray(xb.astype(np.float16))
        wvh = np.ascontiguousarray(wvh.astype(np.float16))
        in_maps.append({
            "xin": xb, "wqk": wqk_full, "wv": wvh,
            "negid": negid_a, "id4": id4_a, "ones2": ones2,
        })

    res = run_bass_kernel_spmd(nc, in_maps, list(range(N_CORES)),
                               trace=bool(_trace))
    if _raw:
        return res

    out = np.empty((B, C, H, W), np.float32)
    for core in range(N_CORES):
        b, h = core // 2, core % 2
        r = res.results[core]
        o_r = r["orow"].astype(np.float32).reshape(W, H, VBW)  # [x, y, :]
        o_c = r["ocol"].astype(np.float32).reshape(H, W, VBW)  # [y, x, :]
        s = o_r[:, :, 256].T + o_c[:, :, 256]                  # [y, x]
        comb = (o_r[:, :, :256].transpose(1, 0, 2) + o_c[:, :, :256]) \
            / s[:, :, None]
        out[b, h * CV:(h + 1) * CV] = comb.transpose(2, 0, 1)

    if _trace:
        return out, res
    return out


# revision 30
# speedup vs baseline: 17.1899x; 1.0116x over previous
"""Criss-cross (CCNet) attention kernel for Trainium2, 8 NeuronCores.

Sharding: core c in 0..7 -> batch b = c//2, value-channel half h = c%2.
Each core computes, for its (b, h): the full joint row+column softmax
attention with 256 of the 512 value/output channels.

Device-side math (per core), H = W = 128, Cqk = 64, Cv = 256:
  - q,k = Wq x, Wk x   (fp16 matmuls)
  - row pass, per row y:  E[i,x] = sum_c k[c,y,i] q[c,y,x]; P = exp(E)
      O[x, 0:256] = P^T V_y, O[x, 256:258] = row-sums of P (two ones
      columns appended to each V slot)
      orow[x*128+y, :] = [O | sums]  (unnormalized bf16, x-major rows)
  - col pass, per col x:  E[j,y] = sum_c k[c,j,x] q[c,y,x]; P = exp(E)
      with diagonal j==y masked to 0 (-6e4*I fp16 matmul); same
      aggregation; ocol rows y-major.
  - v staged to DRAM (bf16) between the passes.
Host combines:  out = (O_r + O_c) / (s_r + s_c)  in fp32.

Precision: projections/energies fp16 (fp32 PSUM accumulation); P, V,
outputs and sums bf16 (P needs fp32-range exponent: energies reach
|e|~50 so exp overflows fp16's range). End-to-end rel err ~6.5e-3.

Performance notes (per core, HW-profiled):
  - 16-bit matmuls stream 1 cyc/row at 2.4 GHz; each matmul also pays
    a ~150ns serial LDWEIGHTS+drain (the PE only hides weight loads
    across disjoint row groups, and mixing tile_positions within one
    PSUM bank wedges the device - HW-bisected - so all matmuls here
    keep partition base 0).
  - softmax runs unnormalized; row-sums ride along as two appended
    ones-columns in the aggregation matmul; normalization happens on
    host (frees ~75us of scalar/vector evacuation + reciprocal work).
  - PSUM->SBUF evacuations batched: one strided [128, 2, 258] bf16
    activation per 2-bank PSUM tile, alternating scalar/vector.
  - orow is stored x-major and ocol y-major so each pass's output DMA
    writes 2-4 KiB contiguous runs per partition line (small DMA
    descriptors otherwise pace the column pass and freeze the HAM
    PE clock-gate at half rate).
  - column gathers prefetch two super-chunks ahead into a 4-group
    transposed-V buffer.
"""

import numpy as np

import concourse.tile as tile
from concourse import bacc, mybir
from concourse.bass_utils import run_bass_kernel_spmd

B, C, H, W = 4, 512, 128, 128
CQK = C // 8          # 64
CV = C // 2           # 256 v channels per core
HW = H * W
N_CORES = 8

F32 = mybir.dt.float32
F16 = mybir.dt.float16
BF16 = mybir.dt.bfloat16
EXP = mybir.ActivationFunctionType.Exp
COPY = mybir.ActivationFunctionType.Copy

_CACHE = {}

VBW = 258   # v buffer width: 256 channels + 2 ones columns


def _build(with_bias):
    nc = bacc.Bacc("TRN2", target_bir_lowering=False, debug=False,
                   num_devices=N_CORES)
    nck = 5 if with_bias else 4   # contraction chunks (last is the bias rows)
    xrows = C + (2 if with_bias else 0)

    xin = nc.dram_tensor("xin", [xrows, HW], F16, kind="ExternalInput").ap()
    wqk = nc.dram_tensor("wqk", [xrows, 128], F16, kind="ExternalInput").ap()
    wv = nc.dram_tensor("wv", [xrows, CV], F16, kind="ExternalInput").ap()
    negid = nc.dram_tensor("negid", [128, 128], F16,
                           kind="ExternalInput").ap()
    id4 = nc.dram_tensor("id4", [128, 512], F16, kind="ExternalInput").ap()
    ones2 = nc.dram_tensor("ones2", [128, 64], BF16, kind="ExternalInput").ap()

    vscr = nc.dram_tensor("vscr", [HW, CV], BF16).ap()
    orow = nc.dram_tensor("orow", [HW, VBW], BF16, kind="ExternalOutput").ap()
    ocol = nc.dram_tensor("ocol", [HW, VBW], BF16, kind="ExternalOutput").ap()

    with tile.TileContext(nc) as tc:
        with (
            tc.tile_pool(name="cst", bufs=1) as cst,
            tc.tile_pool(name="xs", bufs=3) as xsp,
            tc.tile_pool(name="p4", bufs=8) as p4p,
            tc.tile_pool(name="o16", bufs=2) as o16p,
            tc.tile_pool(name="psbig", bufs=2, space="PSUM") as psbigp,
            tc.tile_pool(name="psv", bufs=2, space="PSUM") as psvp,
            tc.tile_pool(name="psO", bufs=2, space="PSUM") as psOp,
        ):
            # x streams as [128, 1024] pair-chunks (chunks 2p, 2p+1) so each
            # partition line is 2 KiB. Prefetch the first two pairs first.
            xpre = {}
            for p in range(2):
                xs0 = []
                for k in range(nck):
                    rows = 128 if k < 4 else 2
                    xt = xsp.tile([128, 1024], F16, tag=f"xs{k}",
                                  name=f"xpre{p}_{k}")
                    nc.sync.dma_start(
                        xt[0:rows, :],
                        xin[k * 128:k * 128 + rows,
                            p * 1024:(p + 1) * 1024])
                    xs0.append(xt)
                xpre[p] = xs0

            # ---- persistent constants / accumulators ----
            WQK = cst.tile([128, nck * 128], F16)
            for k in range(nck):
                rows = 128 if k < 4 else 2
                nc.sync.dma_start(WQK[0:rows, k * 128:k * 128 + 128],
                                  wqk[k * 128:k * 128 + rows, :])
            WV = cst.tile([128, nck * CV], F16)
            for k in range(nck):
                rows = 128 if k < 4 else 2
                nc.sync.dma_start(WV[0:rows, k * CV:k * CV + CV],
                                  wv[k * 128:k * 128 + rows, :])
            NEGID = cst.tile([128, 128], F16)
            nc.sync.dma_start(NEGID[:], negid[:])
            ID4 = cst.tile([128, 512], F16)
            nc.sync.dma_start(ID4[:], id4[:])
            QD = cst.tile([64, HW], F16)
            KD = cst.tile([128, HW], F16)

            # consolidated V buffers: slots of [128, 258], ones columns
            # (256:258 of each slot) loaded once. VTB has 4 groups of 8
            # slots so column gathers can run two pipeline stages ahead.
            VB = cst.tile([128, 8 * VBW], BF16)
            VTB = cst.tile([128, 32 * VBW], BF16)
            nc.sync.dma_start(
                VB[:].rearrange("p (s w) -> p s w", w=VBW)[:, :, 256:258],
                ones2[:, 0:16].rearrange("p (s w) -> p s w", w=2))
            nc.sync.dma_start(
                VTB[:].rearrange("p (s w) -> p s w", w=VBW)[:, :, 256:258],
                ones2[:].rearrange("p (s w) -> p s w", w=2))

            qd_of = QD[:].rearrange("c (y x) -> c y x", x=128)
            kd_of = KD[0:64, :].rearrange("c (y x) -> c y x", x=128)
            vscr_row4 = vscr.rearrange("(g t x) c -> g x t c", t=4, x=128)
            # orow rows x-major (row = x*128 + y), ocol rows y-major
            # (row = y*128 + x): output stores write contiguous runs.
            orow_4 = orow.rearrange("(x g t) c -> g x t c", t=4, g=32)
            ocol_8 = ocol.rearrange("(y g t) c -> g y t c", t=8, g=16)
            vb_slots = VB[:].rearrange("p (s w) -> p s w", w=VBW)
            vtb_slots = VTB[:].rearrange("p (s w) -> p s w", w=VBW)

            # =================== phase A + row pass ===================
            # software pipeline: chunk ch's aggregation tail is emitted
            # after chunk ch+1's head so exp/evac queueing never stalls PE
            row_state = {}

            def row_head(ch):
                csl = slice(ch * 512, (ch + 1) * 512)
                xoff = (ch % 2) * 512
                xsub = slice(xoff, xoff + 512)
                vbase = (ch % 2) * 4
                xs = row_state[("xs", ch // 2)]
                # qk projection for these 512 pixels
                pqk = psbigp.tile([128, 512], F32, tag="psbig")
                for k in range(nck):
                    rows = 128 if k < 4 else 2
                    nc.tensor.matmul(pqk[:],
                                     WQK[0:rows, k * 128:(k + 1) * 128],
                                     xs[k][0:rows, xsub],
                                     start=(k == 0), stop=(k == nck - 1))
                nc.scalar.activation(QD[:, csl], pqk[0:64, :], COPY)
                nc.scalar.activation(KD[64:128, csl], pqk[64:128, :], COPY)
                nc.vector.tensor_copy(KD[0:64, csl], KD[64:128, csl])

                pE = psbigp.tile([128, 512], F32, tag="psbig")
                # v projection, two rows per PSUM bank
                for pair in range(2):
                    pv = psvp.tile([128, 512], F32)
                    for q2 in range(2):
                        yy = pair * 2 + q2
                        xsl = slice(xoff + yy * 128, xoff + yy * 128 + 128)
                        for k in range(nck):
                            rows = 128 if k < 4 else 2
                            nc.tensor.matmul(
                                pv[:, q2 * 256:q2 * 256 + 256],
                                xs[k][0:rows, xsl],
                                WV[0:rows, k * CV:(k + 1) * CV],
                                start=(k == 0), stop=(k == nck - 1))
                    nc.vector.tensor_copy(
                        vb_slots[:, vbase + pair * 2:vbase + pair * 2 + 2,
                                 0:256],
                        pv[:].rearrange("p (b c) -> p b c", c=256))
                for yy in range(4):
                    y = ch * 4 + yy
                    ysl = slice(y * 128, (y + 1) * 128)
                    # row energies E[i, x]
                    nc.tensor.matmul(pE[:, yy * 128:(yy + 1) * 128],
                                     KD[0:64, ysl], QD[:, ysl],
                                     start=True, stop=True)
                # batched v write: 4 rows at once
                nc.sync.dma_start(
                    vscr_row4[ch],
                    vb_slots[:, vbase:vbase + 4, 0:256])
                p4 = p4p.tile([128, 512], BF16)
                nc.scalar.activation(p4[:], pE[:], EXP)
                row_state[ch] = p4
                if ch % 2 == 1:
                    del row_state[("xs", ch // 2)]

            def row_tail(ch):
                vbase = (ch % 2) * 4
                p4 = row_state.pop(ch)
                o16 = o16p.tile([128, 1032], BF16, tag="o16r")
                for half in range(2):
                    pO = psOp.tile([128, 1024], F32)
                    for q2 in range(2):
                        yy = half * 2 + q2
                        osl = slice(q2 * 512, q2 * 512 + VBW)
                        ysl = slice(yy * 128, (yy + 1) * 128)
                        sl = vb_slots[:, vbase + yy:vbase + yy + 1, :]
                        nc.tensor.matmul(pO[:, osl], p4[:, ysl],
                                         sl[:, 0, :],
                                         start=True, stop=True)
                    dst = o16[:, half * 516:half * 516 + 516].rearrange(
                        "p (b w) -> p b w", w=VBW)
                    src = pO[:].rearrange("p (b k) -> p b k",
                                          k=512)[:, :, 0:VBW]
                    if half == 0:
                        nc.scalar.activation(dst, src, COPY)
                    else:
                        nc.vector.tensor_copy(dst, src)
                nc.sync.dma_start(
                    orow_4[ch],
                    o16[:].rearrange("p (t c) -> p t c", c=VBW))

            def load_x(p):
                xs = []
                for k in range(nck):
                    rows = 128 if k < 4 else 2
                    xt = xsp.tile([128, 1024], F16, tag=f"xs{k}")
                    nc.sync.dma_start(
                        xt[0:rows, :],
                        xin[k * 128:k * 128 + rows,
                            p * 1024:(p + 1) * 1024])
                    xs.append(xt)
                row_state[("xs", p)] = xs

            row_state[("xs", 0)] = xpre[0]
            row_state[("xs", 1)] = xpre[1]
            for ch in range(33):
                if ch < 32:
                    if ch % 2 == 0 and ch + 4 < 32:
                        load_x((ch + 4) // 2)
                    row_head(ch)
                if ch >= 1:
                    row_tail(ch - 1)

            # =================== column pass ===================
            vscr_col8 = vscr.rearrange("(j g t) c -> g j t c", t=8, j=128)
            col_state = {}

            def col_gather(sch):
                g = (sch % 4) * 8
                nc.sync.dma_start(vtb_slots[:, g:g + 8, 0:256],
                                  vscr_col8[sch])

            def col_head(sch):
                p4s = []
                for g in range(2):
                    pE = psbigp.tile([128, 512], F32, tag="psbig")
                    for xx in range(4):
                        x = sch * 8 + g * 4 + xx
                        nc.tensor.matmul(pE[:, xx * 128:(xx + 1) * 128],
                                         kd_of[:, :, x], qd_of[:, :, x],
                                         start=(xx == 0), stop=False)
                    # mask the j==y diagonal of all 4 tiles
                    nc.tensor.matmul(pE[:], NEGID[:], ID4[:],
                                     start=False, stop=True)
                    p4 = p4p.tile([128, 512], BF16, tag="p4c")
                    nc.scalar.activation(p4[:], pE[:], EXP)
                    p4s.append(p4)
                col_state[sch] = p4s

            def col_tail(sch):
                vbase = (sch % 4) * 8
                p4s = col_state.pop(sch)
                o16 = o16p.tile([128, 2064], BF16, tag="o16c")
                for g in range(2):
                    p4 = p4s[g]
                    for half in range(2):
                        pO = psOp.tile([128, 1024], F32)
                        for q2 in range(2):
                            xx = half * 2 + q2
                            slot = vbase + g * 4 + xx
                            osl = slice(q2 * 512, q2 * 512 + VBW)
                            xsl = slice(xx * 128, (xx + 1) * 128)
                            sl = vtb_slots[:, slot:slot + 1, :]
                            nc.tensor.matmul(pO[:, osl], p4[:, xsl],
                                             sl[:, 0, :],
                                             start=True, stop=True)
                        oco = (g * 2 + half) * 516
                        dst = o16[:, oco:oco + 516].rearrange(
                            "p (b w) -> p b w", w=VBW)
                        src = pO[:].rearrange("p (b k) -> p b k",
                                              k=512)[:, :, 0:VBW]
                        if (g * 2 + half) % 2 == 0:
                            nc.scalar.activation(dst, src, COPY)
                        else:
                            nc.vector.tensor_copy(dst, src)
                nc.sync.dma_start(
                    ocol_8[sch],
                    o16[:].rearrange("p (t c) -> p t c", c=VBW))

            col_gather(0)
            col_gather(1)
            for sch in range(17):
                if sch < 16:
                    if sch + 2 < 16:
                        col_gather(sch + 2)
                    col_head(sch)
                if sch >= 1:
                    col_tail(sch - 1)

    nc.compile()
    return nc


def _get_nc(with_bias):
    key = bool(with_bias)
    if key not in _CACHE:
        _CACHE[key] = _build(key)
    return _CACHE[key]


def kernel(x, Wq, bq, Wk, bk, Wv, bv, _trace=False, _raw=False):
    x = np.asarray(x, np.float32)
    Wq = np.asarray(Wq, np.float32)
    Wk = np.asarray(Wk, np.float32)
    Wv = np.asarray(Wv, np.float32)
    bq = np.asarray(bq, np.float32)
    bk = np.asarray(bk, np.float32)
    bv = np.asarray(bv, np.float32)

    with_bias = bool(np.any(bq) or np.any(bk) or np.any(bv))
    nc = _get_nc(with_bias)

    import ml_dtypes
    negid_a = np.ascontiguousarray(
        (-6e4 * np.eye(128)).astype(np.float16))
    id4_a = np.ascontiguousarray(
        np.tile(np.eye(128), (1, 4)).astype(np.float16))
    ones2 = np.ones((128, 64), ml_dtypes.bfloat16)
    wqk_full = np.concatenate([Wq.T, Wk.T], axis=1)       # [C, 128]
    if with_bias:
        bias_qk = np.concatenate([bq, bk])[None, :]       # [1, 128]
        wqk_full = np.concatenate(
            [wqk_full, bias_qk, np.zeros_like(bias_qk)], axis=0)
    wqk_full = wqk_full.astype(np.float16)

    in_maps = []
    for core in range(N_CORES):
        b, h = core // 2, core % 2
        xb = x[b].reshape(C, HW)
        wvh = Wv[h * CV:(h + 1) * CV, :].T                # [C, CV]
        if with_bias:
            xb = np.concatenate([xb, np.ones((1, HW), np.float32),
                                 np.zeros((1, HW), np.float32)], axis=0)
            bvh = bv[h * CV:(h + 1) * CV][None, :]
            wvh = np.concatenate([wvh, bvh, np.zeros_like(bvh)], axis=0)
        xb = np.ascontiguousarray(xb.astype(np.float16))
        wvh = np.ascontiguousarray(wvh.astype(np.float16))
        in_maps.append({
            "xin": xb, "wqk": wqk_full, "wv": wvh,
            "negid": negid_a, "id4": id4_a, "ones2": ones2,
        })

    res = run_bass_kernel_spmd(nc, in_maps, list(range(N_CORES)),
                               trace=bool(_trace))
    if _raw:
        return res

    out = np.empty((B, C, H, W), np.float32)
    for core in range(N_CORES):
        b, h = core // 2, core % 2
        r = res.results[core]
        o_r = r["orow"].astype(np.float32).reshape(W, H, VBW)  # [x, y, :]
        o_c = r["ocol"].astype(np.float32).reshape(H, W, VBW)  # [y, x, :]
        s = o_r[:, :, 256].T + o_c[:, :, 256]                  # [y, x]
        comb = (o_r[:, :, :256].transpose(1, 0, 2) + o_c[:, :, :256]) \
            / s[:, :, None]
        out[b, h * CV:(h + 1) * CV] = comb.transpose(2, 0, 1)

    if _trace:
        return out, res
    return out
